# revision 1
# baseline (speedup 1.0000x reference)
"""Trainium2 Bass kernel for CurvSelfAttention (B=2, S=2048, E=1024, H=16).

Sharding: 8 cores = 2 batches x 4 head-quads. Core c handles batch c//4 and
heads [4*(c%4), 4*(c%4)+4). Attention is head-independent, so there are no
collectives; each core gets its batch's hidden states and its heads' weight
row-slices, and returns a [S, 256] slice of the output.

Per-core program (SPMD, identical for all cores):
  1. hidT = hidden.T via PE transposes (fp32 has no DMA-transpose path)
  2. K^T, V ([t, d]), group scales, then Q^T (d on partitions, 2 heads per
     128-partition group), all as matmuls against hidT; K/V first so the
     attention pipeline can overlap the Q-projection tail
  3. scoresT[t, q] = K @ QT per head (t on partitions) -> exp on ScalarE
     (scale=1/8 folded in, no row-max: |scores/8| < ~7 so exp is safe in fp32)
  4. ctxT[d, q] = V_aug.T @ expT accumulated over t, where V_aug carries an
     extra ones-column producing the softmax denominator for free
  5. PE-transpose ctxT back to [q, d], divide by the denominator, DMA out

PSUM budget (8 banks): scores 2x[128,1024] (4) + ctx 2x[65,512] (2) +
out-transpose 2x[128,65] (2). Projections and on-chip transposes allocate
from the scores pool so phases can overlap without bank-lifetime conflicts.
"""

import numpy as np

import concourse.bass as bass
import concourse.mybir as mybir
import concourse.tile as tile
from concourse import bacc, bass_utils
from concourse.masks import make_identity

S = 2048
E = 1024
HL = 4          # heads per core
DH = 64         # head dim
NG = 2          # head groups per core (2 heads each -> 128 partitions)
EJ = E // 128   # 8 contraction tiles
ST = S // 128   # 16 sequence tiles
QB = 512        # projection free-dim block
QBLK = 1024     # attention q block
F32 = mybir.dt.float32
F32R = mybir.dt.float32r

# matmul dtype knobs: False -> exact fp32 (4 cyc/row), True -> float32r
# (full rate at N>=256, reduced-precision multiplies). Walrus requires fp32r
# matmul inputs to be produced as fp32r, so the knob sets the tile dtypes.
R_PROJ = True
R_SCORE = True
R_CTX = True
DT_PROJ = F32R if R_PROJ else F32
DT_SCORE = F32R if R_SCORE else F32
DT_CTX = F32R if R_CTX else F32


def build_program(nc, reps=1, stages="all"):
    hid = nc.dram_tensor("hid", [S, E], F32, kind="ExternalInput")
    wq = nc.dram_tensor("wq", [HL * DH, E], F32, kind="ExternalInput")
    wk = nc.dram_tensor("wk", [HL * DH, E], F32, kind="ExternalInput")
    wv = nc.dram_tensor("wv", [HL * DH, E], F32, kind="ExternalInput")
    ws = nc.dram_tensor("ws", [64, E], F32, kind="ExternalInput")
    bq = nc.dram_tensor("bq", [HL * DH], F32, kind="ExternalInput")
    bk = nc.dram_tensor("bk", [HL * DH], F32, kind="ExternalInput")
    bv = nc.dram_tensor("bv", [HL * DH], F32, kind="ExternalInput")
    bs = nc.dram_tensor("bs", [64], F32, kind="ExternalInput")
    out = nc.dram_tensor("out", [S, HL * DH], F32, kind="ExternalOutput")

    AF = mybir.ActivationFunctionType

    with tile.TileContext(nc) as tc:
        def emit(pfx):
            with (
                tc.tile_pool(name=pfx + "const", bufs=1) as cpool,
                tc.tile_pool(name=pfx + "qkv", bufs=1) as qkv,
                tc.tile_pool(name=pfx + "outp", bufs=10) as outp,
            ):
                pi = [0]

                ident = cpool.tile([128, 128], F32, tag="ident", name=pfx + "ident")
                make_identity(nc, ident[:])

                bqT = cpool.tile([128, NG], F32, tag="bqT", name=pfx + "bqT")
                bkT = cpool.tile([128, NG], F32, tag="bkT", name=pfx + "bkT")
                bsT = cpool.tile([64, 1], F32, tag="bsT", name=pfx + "bsT")
                bv_rep = cpool.tile([128, HL * DH], F32, tag="bv_rep", name=pfx + "bv_rep")
                nc.sync.dma_start(bqT[:], bq.rearrange("(g p) -> p g", p=128))
                nc.sync.dma_start(bkT[:], bk.rearrange("(g p) -> p g", p=128))
                nc.sync.dma_start(bsT[:], bs.rearrange("(g p) -> p g", p=64))
                nc.sync.dma_start(
                    bv_rep[:], bv[None, :].to_broadcast((128, HL * DH))
                )

                ones_col = cpool.tile([128, 1], F32, tag="ones_col", name=pfx + "ones_col")
                nc.vector.memset(ones_col[:], 1.0)
                # 0/1 expansion matrix: emat[j, p] = 1 iff p == 4*(j%32) + r for
                # some r in 0..3 -> (emat.T @ s_val_grp)[p, q] = s_val[p//4, q]
                emat = cpool.tile([64, 128], F32, tag="emat", name=pfx + "emat")
                emat_r = cpool.tile([64, 128], DT_PROJ, tag="emat_r", name=pfx + "emat_r")
                nc.gpsimd.memset(emat[:], 0.0)
                for half in range(2):
                    for r in range(4):
                        nc.gpsimd.affine_select(
                            out=emat[:],
                            in_=emat[:],
                            compare_op=mybir.AluOpType.not_equal,
                            fill=1.0,
                            base=r - 128 * half,
                            pattern=[[-1, 128]],
                            channel_multiplier=4,
                        )
                nc.vector.tensor_copy(emat_r[:], emat[:])

                QT = [qkv.tile([128, S], DT_SCORE, tag=f"QT{g}", name=f"{pfx}QT{g}") for g in range(NG)]
                KT = [qkv.tile([128, S], DT_SCORE, tag=f"KT{g}", name=f"{pfx}KT{g}") for g in range(NG)]
                VA = [qkv.tile([128, HL * 65], DT_CTX, tag=f"VA{t}", name=f"{pfx}VA{t}") for t in range(ST)]

                with (
                    tc.tile_pool(name=pfx + "hidT", bufs=1) as hpool,
                    tc.tile_pool(name=pfx + "wT", bufs=1) as wpool,
                    tc.tile_pool(name=pfx + "spool", bufs=1) as spool,
                    tc.tile_pool(name=pfx + "hraw", bufs=6) as hraw,
                    tc.tile_pool(name=pfx + "wraw", bufs=2) as wraw,
                    tc.tile_pool(name=pfx + "ppsum", bufs=6, space="PSUM") as ppsum,
                ):
                    def pp():
                        pi[0] += 1
                        return ppsum.tile([128, 512], F32, tag="psproj", name=f"{pfx}psp{pi[0]}")

                    # ---- stage B: transposed weights ----
                    wqT = [wpool.tile([128, HL * DH], DT_PROJ, tag=f"wqT{j}", name=f"{pfx}wqT{j}") for j in range(EJ)]
                    wkT = [wpool.tile([128, HL * DH], DT_PROJ, tag=f"wkT{j}", name=f"{pfx}wkT{j}") for j in range(EJ)]
                    wvT = [wpool.tile([128, HL * DH], DT_PROJ, tag=f"wvT{j}", name=f"{pfx}wvT{j}") for j in range(EJ)]
                    wsT = [wpool.tile([128, 64], DT_PROJ, tag=f"wsT{j}", name=f"{pfx}wsT{j}") for j in range(EJ)]
                    for wdram, wT in ((wk, wkT), (wv, wvT), (wq, wqT)):
                        wrs = []
                        for g in range(NG):
                            wr = wraw.tile([128, E], F32, tag="w_raw")
                            nc.sync.dma_start(wr[:], wdram[128 * g : 128 * (g + 1), :])
                            wrs.append(wr)
                        for j in range(EJ):
                            ps = pp()
                            for g in range(NG):
                                nc.tensor.transpose(
                                    ps[:, 128 * g : 128 * (g + 1)],
                                    wrs[g][:, 128 * j : 128 * (j + 1)],
                                    ident[:],
                                )
                            nc.vector.tensor_copy(wT[j][:], ps[:, : HL * DH])
                    wr = wraw.tile([128, E], F32, tag="w_raw")
                    nc.sync.dma_start(wr[0:64, :], ws[:])
                    for j in range(EJ):
                        ps = pp()
                        nc.tensor.transpose(
                            ps[:, 0:64],
                            wr[0:64, 128 * j : 128 * (j + 1)],
                            ident[0:64, 0:64],
                        )
                        nc.vector.tensor_copy(wsT[j][:], ps[:, 0:64])

                    # ---- stage A: hidT[j] = hidden.T e-tile j ----
                    hidT = [hpool.tile([128, S], DT_PROJ, tag=f"hidT{j}", name=f"{pfx}hidT{j}") for j in range(EJ)]
                    for ig in range(ST // 4):
                        hts = []
                        for r4 in range(4):
                            i = ig * 4 + r4
                            ht = hraw.tile([128, E], F32, tag="hid_raw")
                            nc.sync.dma_start(ht[:], hid[128 * i : 128 * (i + 1), :])
                            hts.append(ht)
                        for j in range(EJ):
                            ps = pp()
                            for r4 in range(4):
                                nc.tensor.transpose(
                                    ps[:, 128 * r4 : 128 * (r4 + 1)],
                                    hts[r4][:, 128 * j : 128 * (j + 1)],
                                    ident[:],
                                )
                            nc.vector.tensor_copy(
                                hidT[j][:, 512 * ig : 512 * (ig + 1)], ps[:, 0:512]
                            )

                    # ---- stage D1: K projection (attention needs K/V first) ----
                    for g in range(NG):
                        for qb in range(S // QB):
                            sl = slice(QB * qb, QB * (qb + 1))
                            psk = pp()
                            for j in range(EJ):
                                nc.tensor.matmul(
                                    psk[:, 0:512],
                                    wkT[j][:, 128 * g : 128 * (g + 1)],
                                    hidT[j][:, sl],
                                    start=(j == 0),
                                    stop=(j == EJ - 1),
                                )
                            nc.vector.tensor_scalar_add(
                                KT[g][:, sl], psk[:, 0:512], bkT[:, g : g + 1]
                            )

                    # ---- stage D2: V projection ----
                    for t in range(ST):
                        psv = pp()
                        for j in range(EJ):
                            nc.tensor.matmul(
                                psv[:, 0 : HL * DH],
                                hidT[j][:, 128 * t : 128 * (t + 1)],
                                wvT[j][:],
                                start=(j == 0),
                                stop=(j == EJ - 1),
                            )
                        va = VA[t]
                        va3 = va.rearrange("p (h x) -> p h x", h=HL)
                        nc.vector.tensor_scalar_mul(
                            va3[:, :, 64], ones_col[:, 0:1].to_broadcast((128, HL)), 1.0
                        )
                        nc.vector.tensor_tensor(
                            va3[:, :, 0:64],
                            psv[:, 0 : HL * DH].rearrange("p (h d) -> p h d", h=HL),
                            bv_rep.rearrange("p (h d) -> p h d", h=HL),
                            mybir.AluOpType.add,
                        )

                    # ---- stage D3: group scales s, expanded to per-d rows ----
                    s_val = spool.tile([64, S], DT_PROJ, tag="s_val", name=pfx + "s_val")
                    s_rep = [spool.tile([128, S], F32, tag=f"s_rep{g}", name=f"{pfx}s_rep{g}") for g in range(NG)]
                    for qb in range(S // QB):
                        ps = pp()
                        for j in range(EJ):
                            nc.tensor.matmul(
                                ps[0:64, 0:512],
                                wsT[j][:, 0:64],
                                hidT[j][:, QB * qb : QB * (qb + 1)],
                                start=(j == 0),
                                stop=(j == EJ - 1),
                            )
                        nc.scalar.activation(
                            s_val[:, QB * qb : QB * (qb + 1)],
                            ps[0:64, 0:512],
                            AF.Sigmoid,
                            bias=bsT[:, 0:1],
                        )
                    nc.vector.tensor_scalar(
                        s_val[:], s_val[:], 0.1, 0.95,
                        mybir.AluOpType.mult, mybir.AluOpType.add,
                    )
                    for g in range(NG):
                        for qb in range(S // QB):
                            sl = slice(QB * qb, QB * (qb + 1))
                            pse = pp()
                            nc.tensor.matmul(
                                pse[:, 0:512],
                                emat_r[32 * g : 32 * (g + 1), :],
                                s_val[32 * g : 32 * (g + 1), sl],
                                start=True,
                                stop=True,
                            )
                            nc.scalar.copy(s_rep[g][:, sl], pse[:, 0:512])

                    # ---- stage D4: Q projection with bias + group scaling ----
                    for g in range(NG):
                        for qb in range(S // QB):
                            sl = slice(QB * qb, QB * (qb + 1))
                            psq = pp()
                            for j in range(EJ):
                                nc.tensor.matmul(
                                    psq[:, 0:512],
                                    wqT[j][:, 128 * g : 128 * (g + 1)],
                                    hidT[j][:, sl],
                                    start=(j == 0),
                                    stop=(j == EJ - 1),
                                )
                            nc.vector.tensor_scalar_add(
                                QT[g][:, sl], psq[:, 0:512], bqT[:, g : g + 1]
                            )
                            nc.vector.tensor_tensor(
                                QT[g][:, sl], QT[g][:, sl], s_rep[g][:, sl],
                                mybir.AluOpType.mult,
                            )

                # ---- stage E: attention ----
                if stages == "proj":
                    return
                with (
                    tc.tile_pool(name=pfx + "expT", bufs=24) as expp,
                    tc.tile_pool(name=pfx + "ctxsb", bufs=4) as ctxp,
                    tc.tile_pool(name=pfx + "small", bufs=8) as small,
                    tc.tile_pool(name=pfx + "epsum", bufs=2, space="PSUM") as epsum,
                    tc.tile_pool(name=pfx + "psctx", bufs=2, space="PSUM") as psctx,
                    tc.tile_pool(name=pfx + "pstr", bufs=2, space="PSUM") as pstr,
                ):
                    def bp():
                        pi[0] += 1
                        return epsum.tile([128, QBLK], F32, tag="psbig", name=f"{pfx}psb{pi[0]}")

                    # q processed in 512-blocks; the two heads of a group are
                    # paired: their scores matmuls alternate PE row-groups
                    # (base partitions 0 / 64) into the two halves of one
                    # [128, 1024] PSUM tile, so LDWEIGHTS pulls ahead and the
                    # half-array matmuls overlap, while exp still runs one
                    # [128, 1024] op per t-tile.
                    QW = 512
                    for qblk in range(S // QW):
                        outs = [outp.tile([128, HL * DH], F32, tag="out_sb", name=f"{pfx}out_sb_{qblk}_{ql}") for ql in range(QW // 128)]
                        qsl = slice(QW * qblk, QW * (qblk + 1))
                        for g in range(NG):
                            ets = []
                            for t in range(ST):
                                pss = bp()
                                for sub in range(2):
                                    hb = 64 * sub
                                    nc.tensor.matmul(
                                        pss[:, 512 * sub : 512 * (sub + 1)],
                                        KT[g][hb : hb + 64, 128 * t : 128 * (t + 1)],
                                        QT[g][hb : hb + 64, qsl],
                                        start=True,
                                        stop=True,
                                    )
                                et = expp.tile([128, QBLK], DT_CTX, tag="expT")
                                nc.scalar.activation(et[:], pss[:], AF.Exp, scale=0.125)
                                ets.append(et)
                            for sub in range(2):
                                head = 2 * g + sub
                                psc = psctx.tile([65, 512], F32, tag="psc")
                                for t in range(ST):
                                    nc.tensor.matmul(
                                        psc[:],
                                        VA[t][:, 65 * head : 65 * (head + 1)],
                                        ets[t][:, 512 * sub : 512 * (sub + 1)],
                                        start=(t == 0),
                                        stop=(t == ST - 1),
                                    )
                                cs = ctxp.tile([65, 512], F32, tag="ctx_sb")
                                nc.vector.tensor_copy(cs[:], psc[:])
                                for qs in range(4):
                                    pst = pstr.tile([128, 65], F32, tag="pst")
                                    nc.tensor.transpose(
                                        pst[:],
                                        cs[:, 128 * qs : 128 * (qs + 1)],
                                        ident[0:65, 0:65],
                                    )
                                    rec = small.tile([128, 1], F32, tag="rec")
                                    nc.vector.reciprocal(rec[:], pst[:, 64:65])
                                    nc.vector.tensor_scalar(
                                        outs[qs][:, DH * head : DH * (head + 1)],
                                        pst[:, 0:64],
                                        rec[:, 0:1],
                                        None,
                                        mybir.AluOpType.mult,
                                    )
                        for ql in range(QW // 128):
                            qt = qblk * (QW // 128) + ql
                            nc.sync.dma_start(
                                out[128 * qt : 128 * (qt + 1), :], outs[ql][:]
                            )

        for rep in range(reps):
            emit(f"R{rep}" if reps > 1 else "")
    return nc


_NC = None


def _get_compiled():
    global _NC
    if _NC is None:
        nc = bacc.Bacc(
            "TRN2",
            target_bir_lowering=False,
            debug=False,
            enable_asserts=False,
            num_devices=8,
        )
        build_program(nc)
        nc.compile()
        _NC = nc
    return _NC


def make_in_maps(hidden_states, Wq, bq, Wk, bk, Wv, bv, Ws, bs):
    c32 = lambda a: np.ascontiguousarray(a, dtype=np.float32)
    in_maps = []
    for c in range(8):
        b, hq = divmod(c, 4)
        r = slice(256 * hq, 256 * (hq + 1))
        rs = slice(64 * hq, 64 * (hq + 1))
        in_maps.append(
            {
                "hid": c32(hidden_states[b]),
                "wq": c32(Wq[r]), "bq": c32(bq[r]),
                "wk": c32(Wk[r]), "bk": c32(bk[r]),
                "wv": c32(Wv[r]), "bv": c32(bv[r]),
                "ws": c32(Ws[rs]), "bs": c32(bs[rs]),
            }
        )
    return in_maps


def assemble(results):
    out = np.empty((2, S, 1024), np.float32)
    for c in range(8):
        b, hq = divmod(c, 4)
        out[b, :, 256 * hq : 256 * (hq + 1)] = results[c]["out"]
    return out


def kernel(hidden_states, Wq, bq, Wk, bk, Wv, bv, Ws, bs):
    nc = _get_compiled()
    in_maps = make_in_maps(hidden_states, Wq, bq, Wk, bk, Wv, bv, Ws, bs)
    res = bass_utils.run_bass_kernel_spmd(nc, in_maps, core_ids=list(range(8)))
    return assemble(res.results)

